# revision 22
# baseline (speedup 1.0000x reference)
"""MCSPN Trainium2 kernel v2: guidance convs + softmax gates + 4-step CSPN.

Data-parallel over batch: 8 images -> 8 NeuronCores, one image per core.
Per core, three phases:
  A1: conv3x3 (bf16 matmuls, N=512/row-pair) -> bias+ReLU (ACT, bf16)
      -> conv1x1 (bf16) -> exp (ACT) -> es_cmaj [76ch, H, W] channel-major.
      No DMA transpose scatter (that was the old bottleneck).
  A2: per column x: one PE matmul with stationary es_cmaj[:, :, x] and
      moving [I76 | sum-selector] -> PSUM [128y, 95] = [e.T | s.T]:
      transposes gates to row-major AND computes softmax denominator in
      the same pass. ACT drains e -> e_all bf16; DVE approx-reciprocal
      of s -> ACT cast -> r_all bf16. Gates stay UNNORMALIZED; 1/s is
      applied once per recurrence step.
  B:  4 steps; left/right via free-dim windows of h, up/down via
      sub/super-diagonal shift matmuls -> PSUM -> ACT drain to bf16;
      then 8 whole-image [128, 19*256] DVE ops per step (bf16, 2x mode).
"""
import os
import sys

sys.path.insert(0, "/opt/trn_rl_repo")

import numpy as np

B, CIN, H, W = 8, 256, 128, 256
K = 19
MID = 128
KD = 76  # 4*K channels, stored d-major: c = d*19 + k
EPS = 1e-5
T_STEPS = 4
WP = W + 2  # guarded row width (258)
RG = 8     # feats rows per DMA group
XB = 16    # columns per A2 PSUM tile (4 banks x 4 cols)


def _build():
    import concourse.bacc as bacc
    import concourse.mybir as mybir
    import concourse.tile as tile

    f32 = mybir.dt.float32
    bf16 = mybir.dt.bfloat16
    fp8 = mybir.dt.float8e4
    DR = mybir.MatmulPerfMode.DoubleRow
    Act = mybir.ActivationFunctionType
    Alu = mybir.AluOpType

    nc = bacc.Bacc("TRN2", target_bir_lowering=False)

    feats_d = nc.dram_tensor("feats", [128, 2, H, W], bf16,
                             kind="ExternalInput")
    logits_d = nc.dram_tensor("logits", [H, K, W], bf16, kind="ExternalInput")
    w1t_d = nc.dram_tensor("w1t", [128, 2, 9, MID], bf16,
                           kind="ExternalInput")
    bmid_d = nc.dram_tensor("bmid", [MID, 1], f32, kind="ExternalInput")
    w2t_d = nc.dram_tensor("w2t", [MID, KD], bf16, kind="ExternalInput")
    b2_d = nc.dram_tensor("b2", [KD, 1], f32, kind="ExternalInput")
    selI_d = nc.dram_tensor("selI", [KD, KD], bf16, kind="ExternalInput")
    sup_d = nc.dram_tensor("sup", [128, 128], bf16, kind="ExternalInput")
    sdn_d = nc.dram_tensor("sdn", [128, 128], bf16, kind="ExternalInput")
    out_d = nc.dram_tensor("out", [H, K, W], f32, kind="ExternalOutput")

    with tile.TileContext(nc) as tc:
        with tc.tile_pool(name="persist", bufs=1) as pp:
            es_cmaj = pp.tile([KD, H, W], bf16)
            e_all = pp.tile([128, KD, W], bf16)   # row-major gates, unnorm
            r_all = pp.tile([128, K, W], bf16)    # 1/sum
            h_a = pp.tile([128, K, WP], bf16)
            h_b = pp.tile([128, K, WP], bf16)
            u_all = pp.tile([128, K, W], bf16)
            d_all = pp.tile([128, K, W], bf16)
            w1s = pp.tile([128, 2, 9, MID], bf16)
            w2s = pp.tile([MID, KD], bf16)
            selIs = pp.tile([KD, KD], bf16)
            sups = pp.tile([128, 128], bf16)
            sdns = pp.tile([128, 128], bf16)
            bmids = pp.tile([MID, 1], f32)
            b2s = pp.tile([KD, 1], f32)

            nc.sync.dma_start(out=w1s[:], in_=w1t_d[:])
            nc.sync.dma_start(out=w2s[:], in_=w2t_d[:])
            nc.sync.dma_start(out=selIs[:], in_=selI_d[:])
            nc.sync.dma_start(out=sups[:], in_=sup_d[:])
            nc.sync.dma_start(out=sdns[:], in_=sdn_d[:])
            nc.sync.dma_start(out=bmids[:], in_=bmid_d[:])
            nc.sync.dma_start(out=b2s[:], in_=b2_d[:])

            # h0 = logits with zeroed guard columns
            nc.vector.memset(h_a[:, :, 0:WP:WP - 1], 0.0)
            nc.vector.memset(h_b[:, :, 0:WP:WP - 1], 0.0)
            nc.sync.dma_start(out=h_a[:, :, 1:W + 1], in_=logits_d[:])

            # ================= phase A1: convs =================
            with tc.tile_pool(name="frows", bufs=3) as frp, \
                 tc.tile_pool(name="xrow", bufs=3) as xrp, \
                 tc.tile_pool(name="psA", bufs=3, space="PSUM") as psA, \
                 tc.tile_pool(name="psG", bufs=2, space="PSUM") as psG:
                n_groups = H // RG
                ftiles = []
                for gi in range(n_groups):
                    ft = frp.tile([128, 2, RG, WP], bf16, name=f"ft{gi}",
                                  tag="ft")
                    nc.vector.memset(ft[:, :, :, 0:WP:WP - 1], 0.0)
                    for c in range(2):
                        nc.sync.dma_start(
                            out=ft[:, c, :, 1:W + 1],
                            in_=feats_d[:, c, gi * RG:(gi + 1) * RG, :])
                    ftiles.append(ft)

                    if gi == 0:
                        pairs = [0, 2, 4]
                    elif gi == n_groups - 1:
                        pairs = [8 * gi - 2, 8 * gi, 8 * gi + 2,
                                 8 * gi + 4, 8 * gi + 6]
                    else:
                        pairs = [8 * gi - 2, 8 * gi, 8 * gi + 2, 8 * gi + 4]
                    for y in pairs:
                        acc = psA.tile([MID, 2, W], f32, name="acc")
                        mms = []
                        # ky=1 first: always valid + full N=512 so the
                        # start=True matmul covers every PSUM element
                        for ky in (1, 0, 2):
                            for c in range(2):
                                for kx in range(3):
                                    lw = (c, ky * 3 + kx)
                                    ys, ys2 = y + ky - 1, y + ky
                                    v0 = 0 <= ys < H
                                    v1 = 0 <= ys2 < H
                                    same = (v0 and v1
                                            and ys // RG == ys2 // RG)
                                    if same:
                                        src = ftiles[ys // RG]
                                        mms.append((lw,
                                            src[:, c, ys % RG:ys % RG + 2,
                                                kx:kx + W],
                                            acc[:, :, :]))
                                    else:
                                        if v0:
                                            src = ftiles[ys // RG]
                                            mms.append((lw,
                                                src[:, c, ys % RG, kx:kx + W],
                                                acc[:, 0, :]))
                                        if v1:
                                            src = ftiles[ys2 // RG]
                                            mms.append((lw,
                                                src[:, c, ys2 % RG, kx:kx + W],
                                                acc[:, 1, :]))
                        for i, (lw, rhs, oap) in enumerate(mms):
                            nc.tensor.matmul(
                                out=oap, lhsT=w1s[:, lw[0], lw[1], :],
                                rhs=rhs, start=(i == 0),
                                stop=(i == len(mms) - 1))
                        xr = xrp.tile([MID, 2, W], bf16, name="xr")
                        nc.scalar.activation(xr[:], acc[:], Act.Relu,
                                             bias=bmids[:], scale=1.0)
                        accg = psG.tile([KD, 2, W], f32, name="accg")
                        nc.tensor.matmul(out=accg[:], lhsT=w2s[:], rhs=xr[:],
                                         start=True, stop=True)
                        nc.scalar.activation(es_cmaj[:, y:y + 2, :], accg[:],
                                             Act.Exp, bias=b2s[:], scale=1.0)

            # ====== phase A2: PE transpose + softmax denominator ======
            QW = 64
            with tc.tile_pool(name="psT", bufs=2, space="PSUM") as psT, \
                 tc.tile_pool(name="sq", bufs=2) as sqp:
                for q0 in range(0, W, QW):
                    for x0 in range(q0, q0 + QW, XB):
                        pt = psT.tile([128, 4, 512], f32, name="pt")
                        for b in range(4):
                            for xi in range(4):
                                x = x0 + b * 4 + xi
                                nc.tensor.matmul(
                                    out=pt[:, b, xi * KD:(xi + 1) * KD],
                                    lhsT=es_cmaj[:, :, x], rhs=selIs[:],
                                    start=True, stop=True)
                        ptv = pt[:, :, 0:4 * KD].rearrange(
                            "p b (x c) -> p b x c", x=4)
                        # drain: (b, xi, c) -> e_all[:, c, x0 + b*4 + xi]
                        nc.scalar.activation(
                            e_all[:, :, x0:x0 + XB].rearrange(
                                "p c (b xi) -> p b xi c", b=4),
                            ptv[:], Act.Copy)
                    # softmax denominator for this column quarter
                    sq = sqp.tile([128, K, QW], f32, name="sq")
                    rq = sqp.tile([128, K, QW], f32, name="rq")
                    nc.vector.tensor_tensor(
                        out=sq[:], in0=e_all[:, 0:19, q0:q0 + QW],
                        in1=e_all[:, 19:38, q0:q0 + QW], op=Alu.add)
                    nc.vector.tensor_tensor(
                        out=sq[:], in0=sq[:],
                        in1=e_all[:, 38:57, q0:q0 + QW], op=Alu.add)
                    nc.vector.tensor_tensor(
                        out=sq[:], in0=sq[:],
                        in1=e_all[:, 57:76, q0:q0 + QW], op=Alu.add)
                    nc.vector.reciprocal_approx_fast(out=rq[:], in_=sq[:])
                    nc.scalar.activation(r_all[:, :, q0:q0 + QW], rq[:],
                                         Act.Copy)

            # ================= phase B: recurrence =================
            with tc.tile_pool(name="psU", bufs=2, space="PSUM") as psU, \
                 tc.tile_pool(name="psD", bufs=2, space="PSUM") as psD, \
                 tc.tile_pool(name="tmp", bufs=1) as tp, \
                 tc.tile_pool(name="oq", bufs=2) as oqp:
                t1 = tp.tile([128, K, W], bf16, name="t1")
                t2 = tp.tile([128, K, W], bf16, name="t2")
                t3 = tp.tile([128, K, W], bf16, name="t3")
                cur, nxt = h_a, h_b
                for t in range(T_STEPS):
                    for c0 in range(0, K, 4):
                        kk = min(4, K - c0)
                        ups = psU.tile([128, 4, W], f32, name="ups")
                        dns = psD.tile([128, 4, W], f32, name="dns")
                        for j in range(kk):
                            nc.tensor.matmul(out=ups[:, j, :], lhsT=sups[:],
                                             rhs=cur[:, c0 + j, 1:W + 1],
                                             start=True, stop=True)
                        for j in range(kk):
                            nc.tensor.matmul(out=dns[:, j, :], lhsT=sdns[:],
                                             rhs=cur[:, c0 + j, 1:W + 1],
                                             start=True, stop=True)
                        nc.scalar.activation(u_all[:, c0:c0 + kk, :],
                                             ups[:, 0:kk, :], Act.Copy)
                        nc.scalar.activation(d_all[:, c0:c0 + kk, :],
                                             dns[:, 0:kk, :], Act.Copy)
                    # right-gate mult: low k's on gpsimd (else idle), rest
                    # on DVE; reassociated so drain-dependent ops come last
                    KS = 8
                    nc.gpsimd.tensor_tensor(
                        out=t2[:, 0:KS, :], in0=e_all[:, 19:19 + KS, :],
                        in1=cur[:, 0:KS, 2:WP], op=Alu.mult)
                    nc.vector.tensor_tensor(
                        out=t2[:, KS:19, :], in0=e_all[:, 19 + KS:38, :],
                        in1=cur[:, KS:19, 2:WP], op=Alu.mult)
                    nc.vector.tensor_tensor(out=t1[:], in0=e_all[:, 0:19, :],
                                            in1=cur[:, :, 0:W], op=Alu.mult)
                    nc.vector.tensor_tensor(out=t3[:], in0=e_all[:, 38:57, :],
                                            in1=u_all[:], op=Alu.mult)
                    nc.vector.tensor_tensor(out=t1[:], in0=t1[:], in1=t3[:],
                                            op=Alu.add)
                    nc.vector.tensor_tensor(out=t3[:], in0=e_all[:, 57:76, :],
                                            in1=d_all[:], op=Alu.mult)
                    nc.vector.tensor_tensor(out=t1[:], in0=t1[:], in1=t3[:],
                                            op=Alu.add)
                    nc.vector.tensor_tensor(out=t1[:], in0=t1[:], in1=t2[:],
                                            op=Alu.add)
                    if t < T_STEPS - 1:
                        nc.vector.tensor_tensor(out=nxt[:, :, 1:W + 1],
                                                in0=t1[:], in1=r_all[:],
                                                op=Alu.mult)
                    else:
                        for q0 in range(0, K, 5):
                            qq = min(5, K - q0)
                            oq = oqp.tile([128, 5, W], f32, name="oq")
                            nc.vector.tensor_tensor(
                                out=oq[:, 0:qq, :], in0=t1[:, q0:q0 + qq, :],
                                in1=r_all[:, q0:q0 + qq, :], op=Alu.mult)
                            nc.sync.dma_start(out=out_d[:, q0:q0 + qq, :],
                                              in_=oq[:, 0:qq, :])
                    cur, nxt = nxt, cur

    nc.compile()
    return nc


def _host_prep(feats, logits, w1, gamma, beta, mean, var, w2, b2):
    import ml_dtypes
    bf = ml_dtypes.bfloat16

    feats = np.asarray(feats, dtype=np.float32)
    logits = np.asarray(logits, dtype=np.float32)
    w1 = np.asarray(w1, dtype=np.float32)
    w2 = np.asarray(w2, dtype=np.float32)
    b2 = np.asarray(b2, dtype=np.float32)
    gamma = np.asarray(gamma, dtype=np.float32)
    beta = np.asarray(beta, dtype=np.float32)
    mean = np.asarray(mean, dtype=np.float32)
    var = np.asarray(var, dtype=np.float32)

    inv = gamma / np.sqrt(var + EPS)
    w1f = w1 * inv[:, None, None, None]                 # [MID,CIN,3,3]
    bmid = (beta - mean * inv).astype(np.float32)[:, None]
    w1t = np.ascontiguousarray(
        w1f.transpose(1, 2, 3, 0).reshape(2, 128, 9, MID)
        .transpose(1, 0, 2, 3)).astype(bf)
    # channel permutation to d-major: c' = d*19 + k <- orig k*4 + d
    perm = np.array([(c % K) * 4 + c // K for c in range(KD)])
    w2t = np.ascontiguousarray(w2.reshape(KD, MID)[perm].T).astype(bf)
    b2p = np.ascontiguousarray(b2[perm].astype(np.float32)[:, None])
    selI = np.eye(KD, dtype=np.float32).astype(bf)
    s_up = np.eye(128, k=1, dtype=np.float32).astype(bf)
    s_dn = np.eye(128, k=-1, dtype=np.float32).astype(bf)
    featsb = np.ascontiguousarray(
        feats.reshape(B, 2, 128, H, W).transpose(0, 2, 1, 3, 4)).astype(bf)
    logitsb = np.ascontiguousarray(logits.transpose(0, 2, 1, 3)).astype(bf)

    in_maps = []
    for i in range(B):
        in_maps.append({
            "feats": featsb[i], "logits": logitsb[i],
            "w1t": w1t, "bmid": bmid, "w2t": w2t, "b2": b2p,
            "selI": selI, "sup": s_up, "sdn": s_dn,
        })
    return in_maps


_NC_CACHE = None


def kernel(feats, logits, w1, gamma, beta, mean, var, w2, b2):
    global _NC_CACHE
    from concourse.bass_utils import run_bass_kernel_spmd

    in_maps = _host_prep(feats, logits, w1, gamma, beta, mean, var, w2, b2)

    if _NC_CACHE is None:
        _NC_CACHE = _build()
    nc = _NC_CACHE

    trace = bool(os.environ.get("KTRACE"))
    res = run_bass_kernel_spmd(nc, in_maps, list(range(B)), trace=trace)
    if trace and res.exec_time_ns is not None:
        print(f"HW exec time: {res.exec_time_ns} ns")
    # out is [H, K, W] y-major; untranspose
    out = np.stack([res.results[i]["out"].transpose(1, 0, 2)
                    for i in range(B)], axis=0)
    return out.astype(np.float32)


def _selftest_sim():
    """CoreSim one core against a numpy pipeline reference."""
    from concourse.bass_interp import CoreSim

    rng = np.random.default_rng(0)
    feats = rng.standard_normal((B, CIN, H, W), dtype=np.float32)
    logits = rng.standard_normal((B, K, H, W), dtype=np.float32)
    w1 = (rng.standard_normal((MID, CIN, 3, 3)).astype(np.float32) / 48.0)
    gamma = rng.standard_normal(MID).astype(np.float32) * 0.1 + 1.0
    beta = rng.standard_normal(MID).astype(np.float32) * 0.1
    mean = rng.standard_normal(MID).astype(np.float32) * 0.1
    var = rng.random(MID).astype(np.float32) + 0.5
    w2 = (rng.standard_normal((KD, MID, 1, 1)).astype(np.float32) / 11.3)
    b2 = rng.standard_normal(KD).astype(np.float32) * 0.01

    in_maps = _host_prep(feats, logits, w1, gamma, beta, mean, var, w2, b2)
    nc = _build()
    sim = CoreSim(nc)
    for name, val in in_maps[0].items():
        sim.tensor(name)[:] = val
    sim.simulate()
    got = np.asarray(sim.tensor("out")).transpose(1, 0, 2)

    # numpy reference for image 0
    from scipy.signal import correlate  # noqa: F401  (unused; manual conv)
    inv = gamma / np.sqrt(var + EPS)
    w1f = w1 * inv[:, None, None, None]
    bmid = beta - mean * inv
    f = feats[0]
    xp = np.zeros((MID, H, W), np.float32)
    fpad = np.pad(f, ((0, 0), (1, 1), (1, 1)))
    for ky in range(3):
        for kx in range(3):
            xp += np.einsum('chw,mc->mhw',
                            fpad[:, ky:ky + H, kx:kx + W], w1f[:, :, ky, kx])
    xp = np.maximum(xp + bmid[:, None, None], 0)
    g = np.einsum('mhw,om->ohw', xp, w2.reshape(KD, MID)) \
        + b2[:, None, None]
    e = np.exp(g.reshape(K, 4, H, W))
    s = e.sum(axis=1)
    h = logits[0].copy()
    for t in range(T_STEPS):
        left = np.pad(h, ((0, 0), (0, 0), (1, 0)))[:, :, :W]
        right = np.pad(h, ((0, 0), (0, 0), (0, 1)))[:, :, 1:]
        up = np.pad(h, ((0, 0), (1, 0), (0, 0)))[:, :H, :]
        down = np.pad(h, ((0, 0), (0, 1), (0, 0)))[:, 1:, :]
        h = (e[:, 0] * left + e[:, 1] * right + e[:, 2] * up
             + e[:, 3] * down) / s
    err = np.abs(got - h).max() / np.abs(h).max()
    print(f"sim vs numpy rel err: {err:.5e}")
    assert err < 3e-2, err


if __name__ == "__main__":
    _selftest_sim()


# revision 23
# speedup vs baseline: 1.0222x; 1.0222x over previous
"""MCSPN Trainium2 kernel v2: guidance convs + softmax gates + 4-step CSPN.

Data-parallel over batch: 8 images -> 8 NeuronCores, one image per core.
Per core, three phases:
  A1: conv3x3 (bf16 matmuls, N=512/row-pair) -> bias+ReLU (ACT, bf16)
      -> conv1x1 (bf16) -> exp (ACT) -> es_cmaj [76ch, H, W] channel-major.
      No DMA transpose scatter (that was the old bottleneck).
  A2: per column x: one PE matmul with stationary es_cmaj[:, :, x] and
      moving [I76 | sum-selector] -> PSUM [128y, 95] = [e.T | s.T]:
      transposes gates to row-major AND computes softmax denominator in
      the same pass. ACT drains e -> e_all bf16; DVE approx-reciprocal
      of s -> ACT cast -> r_all bf16. Gates stay UNNORMALIZED; 1/s is
      applied once per recurrence step.
  B:  4 steps; left/right via free-dim windows of h, up/down via
      sub/super-diagonal shift matmuls -> PSUM -> ACT drain to bf16;
      then 8 whole-image [128, 19*256] DVE ops per step (bf16, 2x mode).
"""
import os
import sys

sys.path.insert(0, "/opt/trn_rl_repo")

import numpy as np

B, CIN, H, W = 8, 256, 128, 256
K = 19
MID = 128
KD = 76  # 4*K channels, stored d-major: c = d*19 + k
EPS = 1e-5
T_STEPS = 4
WP = W + 2  # guarded row width (258)
RG = 8     # feats rows per DMA group
XB = 16    # columns per A2 PSUM tile (4 banks x 4 cols)


def _build():
    import concourse.bacc as bacc
    import concourse.mybir as mybir
    import concourse.tile as tile

    f32 = mybir.dt.float32
    bf16 = mybir.dt.bfloat16
    fp8 = mybir.dt.float8e4
    DR = mybir.MatmulPerfMode.DoubleRow
    Act = mybir.ActivationFunctionType
    Alu = mybir.AluOpType

    nc = bacc.Bacc("TRN2", target_bir_lowering=False)

    feats_d = nc.dram_tensor("feats", [128, 2, H, W], bf16,
                             kind="ExternalInput")
    logits_d = nc.dram_tensor("logits", [H, K, W], bf16, kind="ExternalInput")
    w1t_d = nc.dram_tensor("w1t", [128, 2, 9, MID], bf16,
                           kind="ExternalInput")
    bmid_d = nc.dram_tensor("bmid", [MID, 1], f32, kind="ExternalInput")
    w2t_d = nc.dram_tensor("w2t", [MID, KD], bf16, kind="ExternalInput")
    b2_d = nc.dram_tensor("b2", [KD, 1], f32, kind="ExternalInput")
    selI_d = nc.dram_tensor("selI", [KD, KD], bf16, kind="ExternalInput")
    sup_d = nc.dram_tensor("sup", [128, 128], bf16, kind="ExternalInput")
    sdn_d = nc.dram_tensor("sdn", [128, 128], bf16, kind="ExternalInput")
    out_d = nc.dram_tensor("out", [H, K, W], f32, kind="ExternalOutput")

    with tile.TileContext(nc) as tc:
        with tc.tile_pool(name="persist", bufs=1) as pp:
            es_cmaj = pp.tile([KD, H, W], bf16)
            e_all = pp.tile([128, KD, W], bf16)   # row-major gates, unnorm
            r_all = pp.tile([128, K, W], bf16)    # 1/sum
            h_a = pp.tile([128, K, WP], bf16)
            h_b = pp.tile([128, K, WP], bf16)
            u_all = pp.tile([128, K, W], bf16)
            d_all = pp.tile([128, K, W], bf16)
            w1s = pp.tile([128, 2, 9, MID], bf16)
            w2s = pp.tile([MID, KD], bf16)
            selIs = pp.tile([KD, KD], bf16)
            sups = pp.tile([128, 128], bf16)
            sdns = pp.tile([128, 128], bf16)
            bmids = pp.tile([MID, 1], f32)
            b2s = pp.tile([KD, 1], f32)

            nc.sync.dma_start(out=w1s[:], in_=w1t_d[:])
            nc.sync.dma_start(out=w2s[:], in_=w2t_d[:])
            nc.sync.dma_start(out=selIs[:], in_=selI_d[:])
            nc.sync.dma_start(out=sups[:], in_=sup_d[:])
            nc.sync.dma_start(out=sdns[:], in_=sdn_d[:])
            nc.sync.dma_start(out=bmids[:], in_=bmid_d[:])
            nc.sync.dma_start(out=b2s[:], in_=b2_d[:])

            # h0 = logits with zeroed guard columns
            nc.vector.memset(h_a[:, :, 0:WP:WP - 1], 0.0)
            nc.vector.memset(h_b[:, :, 0:WP:WP - 1], 0.0)
            nc.sync.dma_start(out=h_a[:, :, 1:W + 1], in_=logits_d[:])

            # ================= phase A1: convs =================
            with tc.tile_pool(name="frows", bufs=3) as frp, \
                 tc.tile_pool(name="xrow", bufs=3) as xrp, \
                 tc.tile_pool(name="psA", bufs=3, space="PSUM") as psA, \
                 tc.tile_pool(name="psG", bufs=2, space="PSUM") as psG:
                n_groups = H // RG
                ftiles = []
                for gi in range(n_groups):
                    ft = frp.tile([128, 2, RG, WP], bf16, name=f"ft{gi}",
                                  tag="ft")
                    nc.vector.memset(ft[:, :, :, 0:WP:WP - 1], 0.0)
                    for c in range(2):
                        nc.sync.dma_start(
                            out=ft[:, c, :, 1:W + 1],
                            in_=feats_d[:, c, gi * RG:(gi + 1) * RG, :])
                    ftiles.append(ft)

                    if gi == 0:
                        pairs = [0, 2, 4]
                    elif gi == n_groups - 1:
                        pairs = [8 * gi - 2, 8 * gi, 8 * gi + 2,
                                 8 * gi + 4, 8 * gi + 6]
                    else:
                        pairs = [8 * gi - 2, 8 * gi, 8 * gi + 2, 8 * gi + 4]
                    for y in pairs:
                        acc = psA.tile([MID, 2, W], f32, name="acc")
                        mms = []
                        # ky=1 first: always valid + full N=512 so the
                        # start=True matmul covers every PSUM element
                        for ky in (1, 0, 2):
                            for c in range(2):
                                for kx in range(3):
                                    lw = (c, ky * 3 + kx)
                                    ys, ys2 = y + ky - 1, y + ky
                                    v0 = 0 <= ys < H
                                    v1 = 0 <= ys2 < H
                                    same = (v0 and v1
                                            and ys // RG == ys2 // RG)
                                    if same:
                                        src = ftiles[ys // RG]
                                        mms.append((lw,
                                            src[:, c, ys % RG:ys % RG + 2,
                                                kx:kx + W],
                                            acc[:, :, :]))
                                    else:
                                        if v0:
                                            src = ftiles[ys // RG]
                                            mms.append((lw,
                                                src[:, c, ys % RG, kx:kx + W],
                                                acc[:, 0, :]))
                                        if v1:
                                            src = ftiles[ys2 // RG]
                                            mms.append((lw,
                                                src[:, c, ys2 % RG, kx:kx + W],
                                                acc[:, 1, :]))
                        for i, (lw, rhs, oap) in enumerate(mms):
                            nc.tensor.matmul(
                                out=oap, lhsT=w1s[:, lw[0], lw[1], :],
                                rhs=rhs, start=(i == 0),
                                stop=(i == len(mms) - 1))
                        xr = xrp.tile([MID, 2, W], bf16, name="xr")
                        nc.scalar.activation(xr[:], acc[:], Act.Relu,
                                             bias=bmids[:], scale=1.0)
                        accg = psG.tile([KD, 2, W], f32, name="accg")
                        nc.tensor.matmul(out=accg[:], lhsT=w2s[:], rhs=xr[:],
                                         start=True, stop=True)
                        nc.scalar.activation(es_cmaj[:, y:y + 2, :], accg[:],
                                             Act.Exp, bias=b2s[:], scale=1.0)

            # ====== phase A2: PE transpose + softmax denominator ======
            QW = 64
            with tc.tile_pool(name="psT", bufs=2, space="PSUM") as psT, \
                 tc.tile_pool(name="sq", bufs=2) as sqp:
                for q0 in range(0, W, QW):
                    for x0 in range(q0, q0 + QW, XB):
                        pt = psT.tile([128, 4, 512], f32, name="pt")
                        for b in range(4):
                            for xi in range(4):
                                x = x0 + b * 4 + xi
                                nc.tensor.matmul(
                                    out=pt[:, b, xi * KD:(xi + 1) * KD],
                                    lhsT=es_cmaj[:, :, x], rhs=selIs[:],
                                    start=True, stop=True)
                        ptv = pt[:, :, 0:4 * KD].rearrange(
                            "p b (x c) -> p b x c", x=4)
                        # drain: (b, xi, c) -> e_all[:, c, x0 + b*4 + xi]
                        nc.scalar.activation(
                            e_all[:, :, x0:x0 + XB].rearrange(
                                "p c (b xi) -> p b xi c", b=4),
                            ptv[:], Act.Copy)
                    # softmax denominator for this column quarter
                    sq = sqp.tile([128, K, QW], f32, name="sq")
                    rq = sqp.tile([128, K, QW], f32, name="rq")
                    nc.vector.tensor_tensor(
                        out=sq[:], in0=e_all[:, 0:19, q0:q0 + QW],
                        in1=e_all[:, 19:38, q0:q0 + QW], op=Alu.add)
                    nc.vector.tensor_tensor(
                        out=sq[:], in0=sq[:],
                        in1=e_all[:, 38:57, q0:q0 + QW], op=Alu.add)
                    nc.vector.tensor_tensor(
                        out=sq[:], in0=sq[:],
                        in1=e_all[:, 57:76, q0:q0 + QW], op=Alu.add)
                    nc.vector.reciprocal_approx_fast(out=rq[:], in_=sq[:])
                    nc.scalar.activation(r_all[:, :, q0:q0 + QW], rq[:],
                                         Act.Copy)

            # ================= phase B: recurrence =================
            with tc.tile_pool(name="psU", bufs=2, space="PSUM") as psU, \
                 tc.tile_pool(name="psD", bufs=2, space="PSUM") as psD, \
                 tc.tile_pool(name="tmp", bufs=1) as tp, \
                 tc.tile_pool(name="oq", bufs=2) as oqp:
                t1 = tp.tile([128, K, W], bf16, name="t1")
                t2 = tp.tile([128, K, W], bf16, name="t2")
                t3 = tp.tile([128, K, W], bf16, name="t3")
                cur, nxt = h_a, h_b
                for t in range(T_STEPS):
                    for c0 in range(0, K, 4):
                        kk = min(4, K - c0)
                        ups = psU.tile([128, 4, W], f32, name="ups")
                        dns = psD.tile([128, 4, W], f32, name="dns")
                        for j in range(kk):
                            nc.tensor.matmul(out=ups[:, j, :], lhsT=sups[:],
                                             rhs=cur[:, c0 + j, 1:W + 1],
                                             start=True, stop=True)
                        for j in range(kk):
                            nc.tensor.matmul(out=dns[:, j, :], lhsT=sdns[:],
                                             rhs=cur[:, c0 + j, 1:W + 1],
                                             start=True, stop=True)
                        nc.scalar.activation(u_all[:, c0:c0 + kk, :],
                                             ups[:, 0:kk, :], Act.Copy)
                        nc.scalar.activation(d_all[:, c0:c0 + kk, :],
                                             dns[:, 0:kk, :], Act.Copy)
                    nc.vector.tensor_tensor(out=t1[:], in0=e_all[:, 0:19, :],
                                            in1=cur[:, :, 0:W], op=Alu.mult)
                    nc.vector.tensor_tensor(out=t2[:], in0=e_all[:, 19:38, :],
                                            in1=cur[:, :, 2:WP], op=Alu.mult)
                    nc.vector.tensor_tensor(out=t1[:], in0=t1[:], in1=t2[:],
                                            op=Alu.add)
                    nc.vector.tensor_tensor(out=t2[:], in0=e_all[:, 38:57, :],
                                            in1=u_all[:], op=Alu.mult)
                    nc.vector.tensor_tensor(out=t3[:], in0=e_all[:, 57:76, :],
                                            in1=d_all[:], op=Alu.mult)
                    nc.vector.tensor_tensor(out=t2[:], in0=t2[:], in1=t3[:],
                                            op=Alu.add)
                    nc.vector.tensor_tensor(out=t1[:], in0=t1[:], in1=t2[:],
                                            op=Alu.add)
                    if t < T_STEPS - 1:
                        nc.vector.tensor_tensor(out=nxt[:, :, 1:W + 1],
                                                in0=t1[:], in1=r_all[:],
                                                op=Alu.mult)
                    else:
                        for q0 in range(0, K, 5):
                            qq = min(5, K - q0)
                            oq = oqp.tile([128, 5, W], f32, name="oq")
                            nc.vector.tensor_tensor(
                                out=oq[:, 0:qq, :], in0=t1[:, q0:q0 + qq, :],
                                in1=r_all[:, q0:q0 + qq, :], op=Alu.mult)
                            nc.sync.dma_start(out=out_d[:, q0:q0 + qq, :],
                                              in_=oq[:, 0:qq, :])
                    cur, nxt = nxt, cur

    nc.compile()
    return nc


def _host_prep(feats, logits, w1, gamma, beta, mean, var, w2, b2):
    import ml_dtypes
    bf = ml_dtypes.bfloat16

    feats = np.asarray(feats, dtype=np.float32)
    logits = np.asarray(logits, dtype=np.float32)
    w1 = np.asarray(w1, dtype=np.float32)
    w2 = np.asarray(w2, dtype=np.float32)
    b2 = np.asarray(b2, dtype=np.float32)
    gamma = np.asarray(gamma, dtype=np.float32)
    beta = np.asarray(beta, dtype=np.float32)
    mean = np.asarray(mean, dtype=np.float32)
    var = np.asarray(var, dtype=np.float32)

    inv = gamma / np.sqrt(var + EPS)
    w1f = w1 * inv[:, None, None, None]                 # [MID,CIN,3,3]
    bmid = (beta - mean * inv).astype(np.float32)[:, None]
    w1t = np.ascontiguousarray(
        w1f.transpose(1, 2, 3, 0).reshape(2, 128, 9, MID)
        .transpose(1, 0, 2, 3)).astype(bf)
    # channel permutation to d-major: c' = d*19 + k <- orig k*4 + d
    perm = np.array([(c % K) * 4 + c // K for c in range(KD)])
    w2t = np.ascontiguousarray(w2.reshape(KD, MID)[perm].T).astype(bf)
    b2p = np.ascontiguousarray(b2[perm].astype(np.float32)[:, None])
    selI = np.eye(KD, dtype=np.float32).astype(bf)
    s_up = np.eye(128, k=1, dtype=np.float32).astype(bf)
    s_dn = np.eye(128, k=-1, dtype=np.float32).astype(bf)
    featsb = np.ascontiguousarray(
        feats.reshape(B, 2, 128, H, W).transpose(0, 2, 1, 3, 4)).astype(bf)
    logitsb = np.ascontiguousarray(logits.transpose(0, 2, 1, 3)).astype(bf)

    in_maps = []
    for i in range(B):
        in_maps.append({
            "feats": featsb[i], "logits": logitsb[i],
            "w1t": w1t, "bmid": bmid, "w2t": w2t, "b2": b2p,
            "selI": selI, "sup": s_up, "sdn": s_dn,
        })
    return in_maps


_NC_CACHE = None


def kernel(feats, logits, w1, gamma, beta, mean, var, w2, b2):
    global _NC_CACHE
    from concourse.bass_utils import run_bass_kernel_spmd

    in_maps = _host_prep(feats, logits, w1, gamma, beta, mean, var, w2, b2)

    if _NC_CACHE is None:
        _NC_CACHE = _build()
    nc = _NC_CACHE

    trace = bool(os.environ.get("KTRACE"))
    res = run_bass_kernel_spmd(nc, in_maps, list(range(B)), trace=trace)
    if trace and res.exec_time_ns is not None:
        print(f"HW exec time: {res.exec_time_ns} ns")
    # out is [H, K, W] y-major; untranspose
    out = np.stack([res.results[i]["out"].transpose(1, 0, 2)
                    for i in range(B)], axis=0)
    return out.astype(np.float32)


def _selftest_sim():
    """CoreSim one core against a numpy pipeline reference."""
    from concourse.bass_interp import CoreSim

    rng = np.random.default_rng(0)
    feats = rng.standard_normal((B, CIN, H, W), dtype=np.float32)
    logits = rng.standard_normal((B, K, H, W), dtype=np.float32)
    w1 = (rng.standard_normal((MID, CIN, 3, 3)).astype(np.float32) / 48.0)
    gamma = rng.standard_normal(MID).astype(np.float32) * 0.1 + 1.0
    beta = rng.standard_normal(MID).astype(np.float32) * 0.1
    mean = rng.standard_normal(MID).astype(np.float32) * 0.1
    var = rng.random(MID).astype(np.float32) + 0.5
    w2 = (rng.standard_normal((KD, MID, 1, 1)).astype(np.float32) / 11.3)
    b2 = rng.standard_normal(KD).astype(np.float32) * 0.01

    in_maps = _host_prep(feats, logits, w1, gamma, beta, mean, var, w2, b2)
    nc = _build()
    sim = CoreSim(nc)
    for name, val in in_maps[0].items():
        sim.tensor(name)[:] = val
    sim.simulate()
    got = np.asarray(sim.tensor("out")).transpose(1, 0, 2)

    # numpy reference for image 0
    from scipy.signal import correlate  # noqa: F401  (unused; manual conv)
    inv = gamma / np.sqrt(var + EPS)
    w1f = w1 * inv[:, None, None, None]
    bmid = beta - mean * inv
    f = feats[0]
    xp = np.zeros((MID, H, W), np.float32)
    fpad = np.pad(f, ((0, 0), (1, 1), (1, 1)))
    for ky in range(3):
        for kx in range(3):
            xp += np.einsum('chw,mc->mhw',
                            fpad[:, ky:ky + H, kx:kx + W], w1f[:, :, ky, kx])
    xp = np.maximum(xp + bmid[:, None, None], 0)
    g = np.einsum('mhw,om->ohw', xp, w2.reshape(KD, MID)) \
        + b2[:, None, None]
    e = np.exp(g.reshape(K, 4, H, W))
    s = e.sum(axis=1)
    h = logits[0].copy()
    for t in range(T_STEPS):
        left = np.pad(h, ((0, 0), (0, 0), (1, 0)))[:, :, :W]
        right = np.pad(h, ((0, 0), (0, 0), (0, 1)))[:, :, 1:]
        up = np.pad(h, ((0, 0), (1, 0), (0, 0)))[:, :H, :]
        down = np.pad(h, ((0, 0), (0, 1), (0, 0)))[:, 1:, :]
        h = (e[:, 0] * left + e[:, 1] * right + e[:, 2] * up
             + e[:, 3] * down) / s
    err = np.abs(got - h).max() / np.abs(h).max()
    print(f"sim vs numpy rel err: {err:.5e}")
    assert err < 3e-2, err


if __name__ == "__main__":
    _selftest_sim()


# revision 31
# speedup vs baseline: 1.0642x; 1.0411x over previous
"""MCSPN Trainium2 kernel v2: guidance convs + softmax gates + 4-step CSPN.

Data-parallel over batch: 8 images -> 8 NeuronCores, one image per core.
Per core, three phases:
  A1: conv3x3 (bf16 matmuls, N=512/row-pair) -> bias+ReLU (ACT, bf16)
      -> conv1x1 (bf16) -> exp (ACT) -> es_cmaj [76ch, H, W] channel-major.
      No DMA transpose scatter (that was the old bottleneck).
  A2: per column x: one PE matmul with stationary es_cmaj[:, :, x] and
      moving [I76 | sum-selector] -> PSUM [128y, 95] = [e.T | s.T]:
      transposes gates to row-major AND computes softmax denominator in
      the same pass. ACT drains e -> e_all bf16; DVE approx-reciprocal
      of s -> ACT cast -> r_all bf16. Gates stay UNNORMALIZED; 1/s is
      applied once per recurrence step.
  B:  4 steps; left/right via free-dim windows of h, up/down via
      sub/super-diagonal shift matmuls -> PSUM -> ACT drain to bf16;
      then 8 whole-image [128, 19*256] DVE ops per step (bf16, 2x mode).
"""
import os
import sys

sys.path.insert(0, "/opt/trn_rl_repo")

import numpy as np

B, CIN, H, W = 8, 256, 128, 256
K = 19
MID = 128
KD = 76  # 4*K channels, stored d-major: c = d*19 + k
EPS = 1e-5
T_STEPS = 4
WP = W + 2  # guarded row width (258)
RG = 8     # feats rows per DMA group
NB = 6     # y blocks
BR = 22    # rows per block (6*22=132: 4 dummy rows)
NP = NB * K  # 114 partitions in recurrence layout


def _build():
    import concourse.bacc as bacc
    import concourse.mybir as mybir
    import concourse.tile as tile

    f32 = mybir.dt.float32
    bf16 = mybir.dt.bfloat16
    fp8 = mybir.dt.float8e4
    DR = mybir.MatmulPerfMode.DoubleRow
    Act = mybir.ActivationFunctionType
    Alu = mybir.AluOpType

    nc = bacc.Bacc("TRN2", target_bir_lowering=False)

    feats_d = nc.dram_tensor("feats", [128, 2, H, W], bf16,
                             kind="ExternalInput")
    logits_d = nc.dram_tensor("logits", [NP, BR, W], bf16,
                              kind="ExternalInput")
    w1t_d = nc.dram_tensor("w1t", [128, 2, 9, MID], bf16,
                           kind="ExternalInput")
    bmid_d = nc.dram_tensor("bmid", [MID, 1], f32, kind="ExternalInput")
    w2t_d = nc.dram_tensor("w2t", [MID, KD], bf16, kind="ExternalInput")
    b2_d = nc.dram_tensor("b2", [KD, 1], f32, kind="ExternalInput")
    sup_d = nc.dram_tensor("sup", [NP, NP], bf16, kind="ExternalInput")
    sdn_d = nc.dram_tensor("sdn", [NP, NP], bf16, kind="ExternalInput")
    out_d = nc.dram_tensor("out", [NP, BR, W], f32, kind="ExternalOutput")

    with tile.TileContext(nc) as tc:
        with tc.tile_pool(name="persist", bufs=1) as pp:
            es_cmaj = pp.tile([KD, H, W], bf16)
            # y-blocked channel-major recurrence layout: partition
            # p = yb*19 + k (NB=6 blocks x BR=22 rows, 4 dummy rows at
            # the tail of yb=5); up/down become free-dim row offsets.
            e_pk = pp.tile([NP, 4, BR, W], bf16)
            r_pk = pp.tile([NP, BR, W], bf16)
            h_a = pp.tile([NP, BR, WP], bf16)
            h_b = pp.tile([NP, BR, WP], bf16)
            w1s = pp.tile([128, 2, 9, MID], bf16)
            w2s = pp.tile([MID, KD], bf16)
            sups = pp.tile([NP, NP], bf16)
            sdns = pp.tile([NP, NP], bf16)
            bmids = pp.tile([MID, 1], f32)
            b2s = pp.tile([KD, 1], f32)

            nc.sync.dma_start(out=w1s[:], in_=w1t_d[:])
            nc.sync.dma_start(out=w2s[:], in_=w2t_d[:])
            nc.sync.dma_start(out=sups[:], in_=sup_d[:])
            nc.sync.dma_start(out=sdns[:], in_=sdn_d[:])
            nc.sync.dma_start(out=bmids[:], in_=bmid_d[:])
            nc.sync.dma_start(out=b2s[:], in_=b2_d[:])

            # h0 = logits (host pre-blocked, dummy rows zero), guards 0
            nc.vector.memset(h_a[:, :, 0:WP:WP - 1], 0.0)
            nc.vector.memset(h_b[:, :, 0:WP:WP - 1], 0.0)
            nc.sync.dma_start(out=h_a[:, :, 1:W + 1], in_=logits_d[:])
            # dummy-row gates (parts 95.., rows 18..): left=1 keeps the
            # h'=0 chain, others 0. Start at aligned partition 64; the
            # A2 DMAs later overwrite the real-row span of parts 64..94.
            nc.vector.memset(e_pk[64:NP, 0, 18:BR, :], 1.0)
            nc.vector.memset(e_pk[64:NP, 1:4, 18:BR, :], 0.0)

            # ================= phase A1: convs =================
            with tc.tile_pool(name="frows", bufs=3) as frp, \
                 tc.tile_pool(name="xrow", bufs=3) as xrp, \
                 tc.tile_pool(name="psA", bufs=3, space="PSUM") as psA, \
                 tc.tile_pool(name="psG", bufs=2, space="PSUM") as psG:
                n_groups = H // RG
                ftiles = []
                for gi in range(n_groups):
                    ft = frp.tile([128, 2, RG, WP], bf16, name=f"ft{gi}",
                                  tag="ft")
                    nc.vector.memset(ft[:, :, :, 0:WP:WP - 1], 0.0)
                    for c in range(2):
                        nc.sync.dma_start(
                            out=ft[:, c, :, 1:W + 1],
                            in_=feats_d[:, c, gi * RG:(gi + 1) * RG, :])
                    ftiles.append(ft)

                    if gi == 0:
                        pairs = [0, 2, 4]
                    elif gi == n_groups - 1:
                        pairs = [8 * gi - 2, 8 * gi, 8 * gi + 2,
                                 8 * gi + 4, 8 * gi + 6]
                    else:
                        pairs = [8 * gi - 2, 8 * gi, 8 * gi + 2, 8 * gi + 4]
                    for y in pairs:
                        acc = psA.tile([MID, 2, W], f32, name="acc")
                        mms = []
                        # ky=1 first: always valid + full N=512 so the
                        # start=True matmul covers every PSUM element
                        for ky in (1, 0, 2):
                            for c in range(2):
                                for kx in range(3):
                                    lw = (c, ky * 3 + kx)
                                    ys, ys2 = y + ky - 1, y + ky
                                    v0 = 0 <= ys < H
                                    v1 = 0 <= ys2 < H
                                    same = (v0 and v1
                                            and ys // RG == ys2 // RG)
                                    if same:
                                        src = ftiles[ys // RG]
                                        mms.append((lw,
                                            src[:, c, ys % RG:ys % RG + 2,
                                                kx:kx + W],
                                            acc[:, :, :]))
                                    else:
                                        if v0:
                                            src = ftiles[ys // RG]
                                            mms.append((lw,
                                                src[:, c, ys % RG, kx:kx + W],
                                                acc[:, 0, :]))
                                        if v1:
                                            src = ftiles[ys2 // RG]
                                            mms.append((lw,
                                                src[:, c, ys2 % RG, kx:kx + W],
                                                acc[:, 1, :]))
                        for i, (lw, rhs, oap) in enumerate(mms):
                            nc.tensor.matmul(
                                out=oap, lhsT=w1s[:, lw[0], lw[1], :],
                                rhs=rhs, start=(i == 0),
                                stop=(i == len(mms) - 1))
                        xr = xrp.tile([MID, 2, W], bf16, name="xr")
                        nc.scalar.activation(xr[:], acc[:], Act.Relu,
                                             bias=bmids[:], scale=1.0)
                        accg = psG.tile([KD, 2, W], f32, name="accg")
                        nc.tensor.matmul(out=accg[:], lhsT=w2s[:], rhs=xr[:],
                                         start=True, stop=True)
                        nc.scalar.activation(es_cmaj[:, y:y + 2, :], accg[:],
                                             Act.Exp, bias=b2s[:], scale=1.0)

            # ====== phase A2: block-permute gates + softmax denom ======
            # es_cmaj[c=(d,k), y, x] -> e_pk[p=(yb,k), d, y', x] via 24
            # partition-offset SBUF DMAs with fat contiguous lines.
            with tc.tile_pool(name="sq", bufs=1) as sqp:
                for d in range(4):
                    for yb in range(NB):
                        ny = min(BR, H - yb * BR)
                        nc.sync.dma_start(
                            out=e_pk[yb * K:(yb + 1) * K, d, 0:ny, :],
                            in_=es_cmaj[d * K:(d + 1) * K,
                                        yb * BR:yb * BR + ny, :])
                sq = sqp.tile([NP, BR, W], f32, name="sq")
                rq = sqp.tile([NP, BR, W], f32, name="rq")
                nc.vector.tensor_tensor(out=sq[:], in0=e_pk[:, 0],
                                        in1=e_pk[:, 1], op=Alu.add)
                nc.vector.tensor_tensor(out=sq[:], in0=sq[:],
                                        in1=e_pk[:, 2], op=Alu.add)
                nc.vector.tensor_tensor(out=sq[:], in0=sq[:],
                                        in1=e_pk[:, 3], op=Alu.add)
                nc.vector.reciprocal_approx_fast(
                    out=rq[:].rearrange("p y x -> p (y x)"),
                    in_=sq[:].rearrange("p y x -> p (y x)"))
                nc.scalar.activation(r_pk[:], rq[:], Act.Copy)

            # ================= phase B: recurrence =================
            # up/down interior = free-dim row offsets; only the block
            # boundary row needs a partition-shift matmul (1 per dir).
            with tc.tile_pool(name="psB", bufs=2, space="PSUM") as psB, \
                 tc.tile_pool(name="tmp", bufs=1) as tp, \
                 tc.tile_pool(name="oq", bufs=2) as oqp:
                t1 = tp.tile([NP, BR, W], bf16, name="t1")
                t2 = tp.tile([NP, BR, W], bf16, name="t2")
                cur, nxt = h_a, h_b
                for t in range(T_STEPS):
                    up_ps = psB.tile([NP, W], f32, name="up_ps")
                    dn_ps = psB.tile([NP, W], f32, name="dn_ps")
                    nc.tensor.matmul(out=up_ps[:], lhsT=sups[:],
                                     rhs=cur[:, BR - 1, 1:W + 1],
                                     start=True, stop=True)
                    nc.tensor.matmul(out=dn_ps[:], lhsT=sdns[:],
                                     rhs=cur[:, 0, 1:W + 1],
                                     start=True, stop=True)
                    nc.vector.tensor_tensor(out=t1[:], in0=e_pk[:, 0],
                                            in1=cur[:, :, 0:W], op=Alu.mult)
                    nc.vector.tensor_tensor(out=t2[:], in0=e_pk[:, 1],
                                            in1=cur[:, :, 2:WP], op=Alu.mult)
                    nc.vector.tensor_tensor(out=t1[:], in0=t1[:], in1=t2[:],
                                            op=Alu.add)
                    nc.vector.tensor_tensor(out=t2[:, 1:BR, :],
                                            in0=e_pk[:, 2, 1:BR, :],
                                            in1=cur[:, 0:BR - 1, 1:W + 1],
                                            op=Alu.mult)
                    nc.vector.tensor_tensor(out=t2[:, 0, :],
                                            in0=e_pk[:, 2, 0, :],
                                            in1=up_ps[:], op=Alu.mult)
                    nc.vector.tensor_tensor(out=t1[:], in0=t1[:], in1=t2[:],
                                            op=Alu.add)
                    nc.vector.tensor_tensor(out=t2[:, 0:BR - 1, :],
                                            in0=e_pk[:, 3, 0:BR - 1, :],
                                            in1=cur[:, 1:BR, 1:W + 1],
                                            op=Alu.mult)
                    nc.vector.tensor_tensor(out=t2[:, BR - 1, :],
                                            in0=e_pk[:, 3, BR - 1, :],
                                            in1=dn_ps[:], op=Alu.mult)
                    nc.vector.tensor_tensor(out=t1[:], in0=t1[:], in1=t2[:],
                                            op=Alu.add)
                    if t < T_STEPS - 1:
                        nc.vector.tensor_tensor(out=nxt[:, :, 1:W + 1],
                                                in0=t1[:], in1=r_pk[:],
                                                op=Alu.mult)
                    else:
                        hb = BR // 2
                        for q in range(2):
                            r0, r1 = q * hb, BR if q else hb
                            oq = oqp.tile([NP, BR - hb, W], f32, name="oq")
                            nc.vector.tensor_tensor(
                                out=oq[:, 0:r1 - r0, :],
                                in0=t1[:, r0:r1, :],
                                in1=r_pk[:, r0:r1, :], op=Alu.mult)
                            nc.sync.dma_start(out=out_d[:, r0:r1, :],
                                              in_=oq[:, 0:r1 - r0, :])
                    cur, nxt = nxt, cur

    nc.compile()
    return nc


def _host_prep(feats, logits, w1, gamma, beta, mean, var, w2, b2):
    import ml_dtypes
    bf = ml_dtypes.bfloat16

    feats = np.asarray(feats, dtype=np.float32)
    logits = np.asarray(logits, dtype=np.float32)
    w1 = np.asarray(w1, dtype=np.float32)
    w2 = np.asarray(w2, dtype=np.float32)
    b2 = np.asarray(b2, dtype=np.float32)
    gamma = np.asarray(gamma, dtype=np.float32)
    beta = np.asarray(beta, dtype=np.float32)
    mean = np.asarray(mean, dtype=np.float32)
    var = np.asarray(var, dtype=np.float32)

    inv = gamma / np.sqrt(var + EPS)
    w1f = w1 * inv[:, None, None, None]                 # [MID,CIN,3,3]
    bmid = (beta - mean * inv).astype(np.float32)[:, None]
    w1t = np.ascontiguousarray(
        w1f.transpose(1, 2, 3, 0).reshape(2, 128, 9, MID)
        .transpose(1, 0, 2, 3)).astype(bf)
    # channel permutation to d-major: c' = d*19 + k <- orig k*4 + d
    perm = np.array([(c % K) * 4 + c // K for c in range(KD)])
    w2t = np.ascontiguousarray(w2.reshape(KD, MID)[perm].T).astype(bf)
    b2p = np.ascontiguousarray(b2[perm].astype(np.float32)[:, None])
    s_up = np.eye(NP, k=K, dtype=np.float32).astype(bf)
    s_dn = np.eye(NP, k=-K, dtype=np.float32).astype(bf)
    featsb = np.ascontiguousarray(
        feats.reshape(B, 2, 128, H, W).transpose(0, 2, 1, 3, 4)).astype(bf)
    # y-blocked logits: partition yb*K+k holds rows yb*BR..+BR (dummy 0)
    logitsb = np.zeros((B, NP, BR, W), dtype=bf)
    for yb in range(NB):
        ny = min(BR, H - yb * BR)
        logitsb[:, yb * K:(yb + 1) * K, 0:ny, :] = \
            logits[:, :, yb * BR:yb * BR + ny, :].astype(bf)

    in_maps = []
    for i in range(B):
        in_maps.append({
            "feats": featsb[i], "logits": logitsb[i],
            "w1t": w1t, "bmid": bmid, "w2t": w2t, "b2": b2p,
            "sup": s_up, "sdn": s_dn,
        })
    return in_maps


def _unblock_out(res):
    """[NP, BR, W] y-blocked -> [K, H, W]."""
    o = np.zeros((K, H, W), dtype=np.float32)
    for yb in range(NB):
        ny = min(BR, H - yb * BR)
        o[:, yb * BR:yb * BR + ny, :] = res[yb * K:(yb + 1) * K, 0:ny, :]
    return o


_NC_CACHE = None


def kernel(feats, logits, w1, gamma, beta, mean, var, w2, b2):
    global _NC_CACHE
    from concourse.bass_utils import run_bass_kernel_spmd

    in_maps = _host_prep(feats, logits, w1, gamma, beta, mean, var, w2, b2)

    if _NC_CACHE is None:
        _NC_CACHE = _build()
    nc = _NC_CACHE

    trace = bool(os.environ.get("KTRACE"))
    res = run_bass_kernel_spmd(nc, in_maps, list(range(B)), trace=trace)
    if trace and res.exec_time_ns is not None:
        print(f"HW exec time: {res.exec_time_ns} ns")
    out = np.stack([_unblock_out(res.results[i]["out"])
                    for i in range(B)], axis=0)
    return out.astype(np.float32)


def _selftest_sim():
    """CoreSim one core against a numpy pipeline reference."""
    from concourse.bass_interp import CoreSim

    rng = np.random.default_rng(0)
    feats = rng.standard_normal((B, CIN, H, W), dtype=np.float32)
    logits = rng.standard_normal((B, K, H, W), dtype=np.float32)
    w1 = (rng.standard_normal((MID, CIN, 3, 3)).astype(np.float32) / 48.0)
    gamma = rng.standard_normal(MID).astype(np.float32) * 0.1 + 1.0
    beta = rng.standard_normal(MID).astype(np.float32) * 0.1
    mean = rng.standard_normal(MID).astype(np.float32) * 0.1
    var = rng.random(MID).astype(np.float32) + 0.5
    w2 = (rng.standard_normal((KD, MID, 1, 1)).astype(np.float32) / 11.3)
    b2 = rng.standard_normal(KD).astype(np.float32) * 0.01

    in_maps = _host_prep(feats, logits, w1, gamma, beta, mean, var, w2, b2)
    nc = _build()
    sim = CoreSim(nc)
    for name, val in in_maps[0].items():
        sim.tensor(name)[:] = val
    sim.simulate()
    got = _unblock_out(np.asarray(sim.tensor("out")))

    # numpy reference for image 0
    from scipy.signal import correlate  # noqa: F401  (unused; manual conv)
    inv = gamma / np.sqrt(var + EPS)
    w1f = w1 * inv[:, None, None, None]
    bmid = beta - mean * inv
    f = feats[0]
    xp = np.zeros((MID, H, W), np.float32)
    fpad = np.pad(f, ((0, 0), (1, 1), (1, 1)))
    for ky in range(3):
        for kx in range(3):
            xp += np.einsum('chw,mc->mhw',
                            fpad[:, ky:ky + H, kx:kx + W], w1f[:, :, ky, kx])
    xp = np.maximum(xp + bmid[:, None, None], 0)
    g = np.einsum('mhw,om->ohw', xp, w2.reshape(KD, MID)) \
        + b2[:, None, None]
    e = np.exp(g.reshape(K, 4, H, W))
    s = e.sum(axis=1)
    h = logits[0].copy()
    for t in range(T_STEPS):
        left = np.pad(h, ((0, 0), (0, 0), (1, 0)))[:, :, :W]
        right = np.pad(h, ((0, 0), (0, 0), (0, 1)))[:, :, 1:]
        up = np.pad(h, ((0, 0), (1, 0), (0, 0)))[:, :H, :]
        down = np.pad(h, ((0, 0), (0, 1), (0, 0)))[:, 1:, :]
        h = (e[:, 0] * left + e[:, 1] * right + e[:, 2] * up
             + e[:, 3] * down) / s
    err = np.abs(got - h).max() / np.abs(h).max()
    print(f"sim vs numpy rel err: {err:.5e}")
    assert err < 3e-2, err


if __name__ == "__main__":
    _selftest_sim()
